# revision 53
# baseline (speedup 1.0000x reference)
"""Causal multi-head self-attention with RoPE on 8 Trainium2 NeuronCores.

Sharding: data parallel over batch (2) x tensor parallel over heads (4 groups
of 4 heads).  Core c handles batch b = c // 4, head group hg = c % 4.

Per-core dataflow (everything stays in "transposed" [feature, seq] layouts so
no on-device transposes are ever needed):
  QT = wqT.T @ xT   [256, 2048]   (fp16 matmuls, d-contraction on partitions)
  RoPE via a constant shuffle matmul: rot = QT*cosT + (S @ (QT*sinT))
  V  = xT.T @ wvT   [2048, 256] -> fp16, augmented with 64 ones columns per
  head so the PV matmul emits the softmax denominator replicated on
  partitions 64..127 (fast [64,512] DVE reciprocal, no 1-lane ops)
  per head h:
    scores^T[ktile j] = Krot_h[:,128j:128j+128].T @ Qrot_h   (k on partitions)
    expP = exp(scores/8) fp16; diagonal block masked by a 0/1 tri mult (DVE)
    per 512-query chunk: psum[128,512] = sum_j V_aug_j.T @ expP_j
       rows 64..127 are the softmax denominator
    A^T = psum[0:64] * recip(psum[64:128])  -> fp16
  outT_partial = woT.T @ A^T  [1024, 2048] fp32 -> fp16; host sums partials

Scheduling (v4, ~172us vs 191us for v2): every input tensor is DMA'd in
dt-halves split across the sync+gpsimd queues so full-depth column slabs
land as early as possible; wv/x/wq lead so V tiles and the mt0 projections
start the moment data exists.  A dep-free dummy-matmul burst (plus small
packs woven between the first V/Q/K emissions) keeps the HAM clock gate at
8/8 across the DMA-only window.  h0's scores are emitted as single-chunk
[128,<=512] pieces that track the DMA ramp; h1-h3 use 1024-wide pieces.
Score psum rotates 3-deep (sA/sB/sC) and also hosts the rope shuffle psum
and the 1024-wide outproj psum pairs, so the PE is matmul-bound rather than
cast-bound.  pv work is placed in the scalar-bound h1/h3 stretches (pv(0),
pv(1), pv(3,0..2)) so the PE-bound h2 endgame carries only pv(2)/pv(3,3) +
outproj; pv(2,3) pre-accumulates k-tiles 0..11 before the last exp, and
outproj(2) runs after the final score piece so its matmuls cover the last
exp lull (no HAM drop before the tail).  psum->fp16 casts alternate between
the vector and scalar engines; output DMA goes out in 4-feature-tile pieces
alternating queues (finer on the last slice).  The engines execute their
queues in emission order, so the emission order below IS the schedule.
PE ~100% busy from ~30us to ~160us; remaining time is the DMA-bound ramp
(~21us of input at ~350GB/s + ~9us queue spin-up) and ~16us of fixed NEFF
preamble/epilogue.  Cross-partition-base DVE operand pairs fail on HW
(lanes have no cross-lane path) - the pv denominator is copied to base-0
partitions before reciprocal_approx_fast.
"""

import numpy as np

import concourse.bass as bass
import concourse.mybir as mybir
import concourse.tile as tile
from concourse import bacc
from concourse.bass_utils import run_bass_kernel_spmd

F32 = mybir.dt.float32
F16 = mybir.dt.float16

B, S, D, H, DH = 2, 2048, 1024, 16, 64
ROPE_THETA = 10000.0
NCORE = 8
HPG = 4          # heads per group (per core)
P = 128
NKT = S // P     # 16 k-tiles
NQC = S // 512   # 4 query chunks

# expP storage: k-tile j's columns start at global q = 128*j; width below.
_W = [S - P * j for j in range(NKT)]
_OFF = np.concatenate([[0], np.cumsum(_W)]).astype(int)
EXP_TOT = int(_OFF[-1])  # 17408 columns of fp16 -> 34KB/partition


def build_program():
    nc = bacc.Bacc(
        "TRN2", target_bir_lowering=False, debug=False, num_devices=NCORE
    )

    xT = nc.dram_tensor("xT", [D, S], F16, kind="ExternalInput")
    wqT = nc.dram_tensor("wqT", [D, 256], F16, kind="ExternalInput")
    wkT = nc.dram_tensor("wkT", [D, 256], F16, kind="ExternalInput")
    wvT = nc.dram_tensor("wvT", [D, 256], F16, kind="ExternalInput")
    woT = nc.dram_tensor("woT", [256, D], F16, kind="ExternalInput")
    cosT = nc.dram_tensor("cosT", [P, S], F16, kind="ExternalInput")
    sinT = nc.dram_tensor("sinT", [P, S], F16, kind="ExternalInput")
    ST = nc.dram_tensor("ST", [P, P], F16, kind="ExternalInput")
    trimask = nc.dram_tensor("trimask", [P, P], F16, kind="ExternalInput")

    outT = nc.dram_tensor("outT", [D, S], F16, kind="ExternalOutput")

    with tile.TileContext(nc) as tc:
        with (
            tc.tile_pool(name="const", bufs=1) as cpool,
            tc.tile_pool(name="qkv", bufs=1) as qkv,
            tc.tile_pool(name="psum", bufs=1, space="PSUM") as psum,
            tc.tile_pool(name="agp", bufs=6) as agp,
            tc.tile_pool(name="atmp", bufs=3) as atmp,
        ):
            tri_sb = cpool.tile([P, P], F16, tag="tri")
            wo_sb = cpool.tile([P, 2, D], F16, tag="wo")
            st_sb = cpool.tile([P, P], F16, tag="st")
            cos_sb = cpool.tile([P, S], F16, tag="cos")
            sin_sb = cpool.tile([P, S], F16, tag="sin")
            wq_sb = cpool.tile([P, 8, 256], F16, tag="wq")
            wk_sb = cpool.tile([P, 8, 256], F16, tag="wk")
            wv_sb = cpool.tile([P, 8, 256], F16, tag="wv")
            xt_sb = cpool.tile([P, 8, S], F16, tag="xt")

            qrot = qkv.tile([P, 2, S], F16, tag="qrot")
            krot = qkv.tile([P, 2, S], F16, tag="krot")
            # per k-tile, per head: [V (64) | ones (64)] so PV emits the
            # denominator replicated on partitions 64..127
            v_sb = qkv.tile([P, NKT, HPG, P], F16, tag="v")
            at_sb = qkv.tile([P, 2, S], F16, tag="at")
            eps = [
                qkv.tile([P, EXP_TOT], F16, tag=f"expp{i}", name=f"ep{i}")
                for i in range(2)
            ]
            # head 3's exp buffer recycles the x buffer: x is fully consumed
            # (mt1 projections + V) before h3's first score piece writes it
            eps.append(cpool.tile([P, EXP_TOT], F16, tag="xt", name="ep2"))
            EPMAP = {0: 0, 1: 1, 2: 0, 3: 2}

            # ---- PE warm-up: dummy matmuls spanning the DMA-only window so
            # the HAM clock gate is at 8/8 when the first real operands land.
            warm_w = cpool.tile([P, P], F16, tag="warm_w")
            warm_m = cpool.tile([P, 256], F16, tag="warm_m")
            nc.vector.memset(warm_w[:], 0.0)
            nc.vector.memset(warm_m[:], 0.0)
            # shares the sC slot: all weave points precede the first score
            # piece, so nothing live is clobbered and no cross-engine waits
            wp = psum.tile([P, 256], F32, tag="sC", bufs=1, name="warm")

            def emit_warm(n):
                # dep-free dummy matmuls: bridge DMA-wait gaps so the HAM
                # clock gate never sees an idle PE during the ramp
                for _ in range(n):
                    nc.tensor.matmul(
                        wp[:], warm_w[:], warm_m[:], start=True, stop=True
                    )

            emit_warm(48)

            # ---- input DMA: every tensor split in dt-halves across BOTH
            # queues (sync gets dt 0:4, gpsimd dt 4:8) so full-depth column
            # slabs complete as early as possible; consts interleaved.
            xr = xT.rearrange("(n p) m -> p n m", p=P)
            wvr = wvT.rearrange("(n p) m -> p n m", p=P)
            wqr = wqT.rearrange("(n p) m -> p n m", p=P)
            wkr = wkT.rearrange("(n p) m -> p n m", p=P)
            wor = woT.rearrange("(n p) m -> p n m", p=P)
            nc.sync.dma_start(out=wv_sb[:, 0:4], in_=wvr[:, 0:4])
            nc.gpsimd.dma_start(out=wv_sb[:, 4:8], in_=wvr[:, 4:8])
            nc.sync.dma_start(out=xt_sb[:, 0:4, 0:512], in_=xr[:, 0:4, 0:512])
            nc.gpsimd.dma_start(out=xt_sb[:, 4:8, 0:512], in_=xr[:, 4:8, 0:512])
            nc.sync.dma_start(out=wq_sb[:, 0:4], in_=wqr[:, 0:4])
            nc.gpsimd.dma_start(out=wq_sb[:, 4:8], in_=wqr[:, 4:8])
            nc.sync.dma_start(out=sin_sb[:, 0:1024], in_=sinT[:, 0:1024])
            nc.gpsimd.dma_start(out=st_sb[:], in_=ST[:, :])
            nc.gpsimd.dma_start(out=tri_sb[:], in_=trimask[:, :])
            nc.gpsimd.dma_start(out=cos_sb[:, 0:1024], in_=cosT[:, 0:1024])
            nc.sync.dma_start(out=wk_sb[:, 0:4], in_=wkr[:, 0:4])
            nc.gpsimd.dma_start(out=wk_sb[:, 4:8], in_=wkr[:, 4:8])
            nc.sync.dma_start(out=xt_sb[:, 0:4, 512:1024], in_=xr[:, 0:4, 512:1024])
            nc.gpsimd.dma_start(out=xt_sb[:, 4:8, 512:1024], in_=xr[:, 4:8, 512:1024])
            nc.sync.dma_start(out=xt_sb[:, 0:4, 1024:1536], in_=xr[:, 0:4, 1024:1536])
            nc.gpsimd.dma_start(out=xt_sb[:, 4:8, 1024:1536], in_=xr[:, 4:8, 1024:1536])
            nc.sync.dma_start(out=sin_sb[:, 1024:2048], in_=sinT[:, 1024:2048])
            nc.gpsimd.dma_start(out=cos_sb[:, 1024:2048], in_=cosT[:, 1024:2048])
            nc.sync.dma_start(out=xt_sb[:, 0:4, 1536:2048], in_=xr[:, 0:4, 1536:2048])
            nc.gpsimd.dma_start(out=xt_sb[:, 4:8, 1536:2048], in_=xr[:, 4:8, 1536:2048])
            nc.sync.dma_start(out=wo_sb[:, 0:1], in_=wor[:, 0:1])
            nc.gpsimd.dma_start(out=wo_sb[:, 1:2], in_=wor[:, 1:2])

            # ---------------- emission helpers -----------------
            def emit_proj(mt, sc, which):
                # one projection (q or k) + rope for one (mt, sc) pair
                ssl = bass.ts(sc, 512)
                w_sb, rot = (
                    (wq_sb, qrot) if which == "q" else (wk_sb, krot)
                )
                pp = psum.tile([P, 512], F32, tag="proj", bufs=2)
                for dt in range(8):
                    nc.tensor.matmul(
                        pp[:],
                        w_sb[:, dt, P * mt:P * (mt + 1)],
                        xt_sb[:, dt, ssl],
                        start=(dt == 0),
                        stop=(dt == 7),
                    )
                t_s = atmp.tile([P, 512], F16, tag="ts", bufs=3)
                nc.vector.tensor_tensor(
                    out=t_s[:], in0=pp[:], in1=sin_sb[:, ssl],
                    op=mybir.AluOpType.mult,
                )
                sh = _score_psum(512)
                nc.tensor.matmul(
                    sh[:], st_sb[:], t_s[:], start=True, stop=True
                )
                nc.vector.tensor_tensor(
                    out=rot[:, mt, ssl], in0=pp[:],
                    in1=cos_sb[:, ssl], op=mybir.AluOpType.mult,
                )
                nc.vector.tensor_tensor(
                    out=rot[:, mt, ssl], in0=rot[:, mt, ssl],
                    in1=sh[:], op=mybir.AluOpType.add,
                )

            def emit_v(st):
                # V projection for one 128-row seq tile -> fp16 V_aug
                vp = psum.tile([P, 256], F32, tag="proj", bufs=2, name="vp")
                for dt in range(8):
                    nc.tensor.matmul(
                        vp[:],
                        xt_sb[:, dt, P * st:P * (st + 1)],
                        wv_sb[:, dt, :],
                        start=(dt == 0),
                        stop=(dt == 7),
                    )
                # first tiles cast on the DVE (idle before the rope chains
                # begin); later ones on the scalar engine (idle pre-exp) so
                # the DVE stays out of the mid-ramp critical path
                if st < 4:
                    nc.vector.tensor_copy(
                        out=v_sb[:, st, :, 0:DH],
                        in_=vp.rearrange("p (h d) -> p h d", h=HPG),
                    )
                else:
                    nc.scalar.activation(
                        out=v_sb[:, st, :, 0:DH],
                        in_=vp.rearrange("p (h d) -> p h d", h=HPG),
                        func=mybir.ActivationFunctionType.Copy,
                    )

            piece_ctr = [0]

            def _score_psum(width):
                tag = ("sA", "sB", "sC")[piece_ctr[0] % 3]
                piece_ctr[0] += 1
                return psum.tile([P, width], F32, tag=tag, bufs=1, name="sp")

            def emit_piece(h, j, c, tri="defer"):
                # single-chunk score piece: head h, k-tile j, query chunk c
                ep = eps[EPMAP[h]]
                th, bs = h // 2, 64 * (h % 2)
                qh = qrot[bs:bs + 64, th, :]
                kh = krot[bs:bs + 64, th, :]
                off = int(_OFF[j])
                r = j % 4
                ls = P * r if j // 4 == c else 0
                q0 = 512 * c + ls
                w = 512 * (c + 1) - q0
                sp = _score_psum(512)
                nc.tensor.matmul(
                    sp[:, ls:ls + w],
                    kh[:, P * j:P * (j + 1)],
                    qh[:, q0:q0 + w],
                    start=True, stop=True,
                )
                nc.scalar.activation(
                    out=ep[:, off + q0 - P * j:off + q0 - P * j + w],
                    in_=sp[:, ls:ls + w],
                    func=mybir.ActivationFunctionType.Exp,
                    scale=0.125,
                )
                if j // 4 == c:
                    if tri == "defer":
                        # deferred out of the DVE-choked ramp; flushed (on
                        # gpsimd) before the first pv consumer
                        deferred_tri.append((ep, off))
                    else:
                        nc.vector.tensor_tensor(
                            out=ep[:, off:off + P], in0=ep[:, off:off + P],
                            in1=tri_sb[:], op=mybir.AluOpType.mult,
                        )

            deferred_tri = []

            def flush_tri():
                # on gpsimd: idle after input DMA issue, and this keeps the
                # DVE out of the ramp critical path
                for ep, off in deferred_tri:
                    nc.gpsimd.tensor_tensor(
                        out=ep[:, off:off + P], in0=ep[:, off:off + P],
                        in1=tri_sb[:], op=mybir.AluOpType.mult,
                    )
                del deferred_tri[:]

            def emit_scores(h, j):
                # all remaining chunks for (h, j) in up-to-1024-wide pieces
                ep = eps[EPMAP[h]]
                th, bs = h // 2, 64 * (h % 2)
                qh = qrot[bs:bs + 64, th, :]
                kh = krot[bs:bs + 64, th, :]
                c0, r = j // 4, j % 4
                off = int(_OFF[j])
                cs = list(range(c0, 4))
                for gi in range(0, len(cs), 2):
                    grp = cs[gi:gi + 2]
                    ca = grp[0]
                    sp = _score_psum(1024)
                    for c in grp:
                        loc = 512 * (c - ca)
                        if c == c0:
                            nc.tensor.matmul(
                                sp[:, loc + 128 * r:loc + 512],
                                kh[:, P * j:P * (j + 1)],
                                qh[:, 512 * c + 128 * r:512 * (c + 1)],
                                start=True, stop=True,
                            )
                        else:
                            nc.tensor.matmul(
                                sp[:, loc:loc + 512],
                                kh[:, P * j:P * (j + 1)],
                                qh[:, 512 * c:512 * (c + 1)],
                                start=True, stop=True,
                            )
                    ls = 128 * r if ca == c0 else 0
                    qstart = 512 * ca + ls
                    w = 512 * (grp[-1] + 1) - qstart
                    eo = off + qstart - 128 * j
                    nc.scalar.activation(
                        out=ep[:, eo:eo + w],
                        in_=sp[:, ls:ls + w],
                        func=mybir.ActivationFunctionType.Exp,
                        scale=0.125,
                    )
                    if gi == 0:
                        nc.vector.tensor_tensor(
                            out=ep[:, off:off + P], in0=ep[:, off:off + P],
                            in1=tri_sb[:], op=mybir.AluOpType.mult,
                        )

            pv_pend = {}

            def emit_pv(h, c, j_lo=0, j_hi=None, finish=True):
                ep = eps[EPMAP[h]]
                th, bs = h // 2, 64 * (h % 2)
                last_j = 4 * c + 3
                if j_hi is None:
                    j_hi = last_j
                if (h, c) in pv_pend:
                    pv = pv_pend.pop((h, c))
                else:
                    pv = psum.tile([P, 512], F32, tag="proj", bufs=2, name="pv")
                for j in range(j_lo, j_hi + 1):
                    off = int(_OFF[j])
                    if j // 4 == c:
                        rr = j % 4
                        n = 512 - 128 * rr
                        nc.tensor.matmul(
                            pv[:, 128 * rr:512],
                            v_sb[:, j, h, :],
                            ep[:, off:off + n],
                            start=(j == 0), stop=(j == last_j),
                        )
                    else:
                        st_col = off + 512 * c - 128 * j
                        nc.tensor.matmul(
                            pv[:, :],
                            v_sb[:, j, h, :],
                            ep[:, st_col:st_col + 512],
                            start=(j == 0), stop=(j == last_j),
                        )
                if not finish:
                    pv_pend[(h, c)] = pv
                    return
                den = atmp.tile([DH, 512], F32, tag="den", bufs=2)
                nc.vector.tensor_copy(out=den[:], in_=pv[DH:P, :])
                recip = atmp.tile([DH, 512], F32, tag="recip", bufs=2)
                nc.vector.reciprocal_approx_fast(out=recip[:], in_=den[:])
                nc.vector.tensor_tensor(
                    out=at_sb[bs:bs + 64, th, 512 * c:512 * (c + 1)],
                    in0=pv[0:DH, :], in1=recip[:],
                    op=mybir.AluOpType.mult,
                )

            outTr = outT.rearrange("(n p) m -> p n m", p=P)

            def outproj_slice(sc, cast="vector", split_dma=False):
                # outT_partial[:, sc] = sum over the 256 LOCAL attention
                # dims (this core's 4 heads); host sums the partials.
                # Two output-feature tiles per [P,1024] psum (on the 3-deep
                # score rotation, so the PE is matmul-bound, not cast-bound);
                # one cast per pair; one DMA per 4 tiles, alternating queues.
                ssl = bass.ts(sc, 512)
                for grp in range(2):
                    ob = agp.tile([P, 4, 512], F16, tag="ob", name="ob")
                    for half in range(2):
                        po = _score_psum(1024)
                        for oi in range(2):
                            ot = 4 * grp + 2 * half + oi
                            osl = bass.ts(ot, P)
                            for ct in range(2):
                                nc.tensor.matmul(
                                    po[:, 512 * oi:512 * (oi + 1)],
                                    wo_sb[:, ct, osl],
                                    at_sb[:, ct, ssl],
                                    start=(ct == 0), stop=(ct == 1),
                                )
                        use_scalar = (
                            cast == "scalar"
                            or (cast == "both" and half % 2 == 1)
                        )
                        if use_scalar:
                            nc.scalar.activation(
                                out=ob[:, 2 * half:2 * half + 2],
                                in_=po.rearrange("p (a b) -> p a b", a=2),
                                func=mybir.ActivationFunctionType.Copy,
                            )
                        else:
                            nc.vector.tensor_copy(
                                out=ob[:, 2 * half:2 * half + 2],
                                in_=po.rearrange("p (a b) -> p a b", a=2),
                            )
                        if split_dma:
                            # finer pieces for the kernel tail: both queues
                            # stream the final slice out in parallel
                            o0 = 4 * grp + 2 * half
                            q = nc.sync if (2 * grp + half) % 2 == 0 else nc.gpsimd
                            q.dma_start(
                                out=outTr[:, o0:o0 + 2, ssl],
                                in_=ob[:, 2 * half:2 * half + 2],
                            )
                    if split_dma:
                        continue
                    if grp == 0:
                        nc.sync.dma_start(
                            out=outTr[:, 0:4, ssl], in_=ob[:]
                        )
                    else:
                        nc.gpsimd.dma_start(
                            out=outTr[:, 4:8, ssl], in_=ob[:]
                        )

            # ---------------- emission schedule -----------------
            # Engines execute their queues in emission order, so this order IS
            # the schedule.  Constraints: the scalar-engine exp stream must
            # start ASAP and never starve; the PE must always have
            # exp-independent filler (V proj, projections, pv of older heads)
            # between score pieces; ep buffers are shared h0/h2, so all pv(0)
            # precede h2 scores; ep2 aliases x, so h3 scores follow the last
            # x consumer (V15 / mt1 sc3 projections).

            # -- ramp (DMA-gated): V tiles as x columns land, mt0 Q/K,
            # granular h0 score pieces chunk-by-chunk
            emit_v(0)
            emit_warm(6)
            emit_v(1)
            emit_warm(6)
            emit_proj(0, 0, "q")
            emit_warm(6)
            emit_v(2)
            emit_warm(4)
            emit_v(3)
            emit_warm(4)
            emit_proj(0, 0, "k")
            for j in range(4):
                emit_piece(0, j, 0)
            emit_proj(0, 1, "q")
            emit_proj(0, 1, "k")
            emit_piece(0, 0, 1)
            emit_piece(0, 1, 1)
            emit_v(4)
            emit_piece(0, 2, 1)
            emit_piece(0, 3, 1)
            emit_v(5)
            emit_piece(0, 4, 1)
            emit_piece(0, 5, 1)
            emit_v(6)
            emit_piece(0, 6, 1)
            emit_piece(0, 7, 1)
            emit_v(7)
            emit_proj(0, 2, "q")
            emit_proj(0, 2, "k")
            emit_piece(0, 0, 2)
            emit_piece(0, 1, 2)
            emit_v(8)
            emit_piece(0, 2, 2)
            emit_piece(0, 3, 2)
            emit_v(9)
            emit_piece(0, 4, 2)
            emit_piece(0, 5, 2)
            emit_v(10)
            emit_piece(0, 6, 2)
            emit_piece(0, 7, 2)
            emit_v(11)
            emit_piece(0, 8, 2)
            emit_piece(0, 9, 2)
            emit_proj(0, 3, "q")
            emit_piece(0, 10, 2)
            emit_piece(0, 11, 2)
            emit_proj(0, 3, "k")
            emit_piece(0, 0, 3)
            emit_piece(0, 1, 3)
            emit_v(12)
            emit_piece(0, 2, 3)
            emit_piece(0, 3, 3)
            emit_v(13)
            emit_piece(0, 4, 3)
            emit_piece(0, 5, 3)
            emit_v(14)
            emit_piece(0, 6, 3)
            emit_piece(0, 7, 3)
            emit_v(15)
            emit_piece(0, 8, 3)
            emit_piece(0, 9, 3)
            emit_proj(1, 0, "q")
            emit_piece(0, 10, 3)
            emit_piece(0, 11, 3)
            emit_proj(1, 0, "k")
            emit_piece(0, 12, 3)
            emit_piece(0, 13, 3)
            emit_proj(1, 1, "q")
            emit_piece(0, 14, 3)
            emit_piece(0, 15, 3)
            emit_proj(1, 1, "k")

            # -- h1 pieces + rest of mt1 + pv(0); deferred DVE housekeeping
            # (h0 diagonal masks + the V ones-columns) runs here where the
            # vector engine has slack
            nc.gpsimd.memset(v_sb[:, :, :, DH:P], 1.0)
            flush_tri()
            emit_scores(1, 0)
            emit_proj(1, 2, "q")
            emit_scores(1, 1)
            emit_proj(1, 2, "k")
            emit_scores(1, 2)
            emit_pv(0, 0)
            emit_scores(1, 3)
            emit_proj(1, 3, "q")
            emit_scores(1, 4)
            emit_proj(1, 3, "k")
            emit_scores(1, 5)
            emit_pv(0, 1)
            emit_scores(1, 6)
            emit_scores(1, 7)
            emit_pv(0, 2)
            emit_scores(1, 8)
            emit_scores(1, 9)
            emit_pv(0, 3)
            emit_scores(1, 10)
            emit_scores(1, 11)
            emit_pv(1, 0)
            emit_scores(1, 12)
            emit_scores(1, 13)
            emit_pv(1, 1)
            emit_scores(1, 14)
            emit_scores(1, 15)

            # -- h3 pieces (own ep buffer = x alias; x fully consumed);
            # h3 is scalar-bound, so it absorbs pv(1,2/3) and pv(3,0..2):
            # h1-dependent fillers front-loaded, h3-dependent ones late
            # enough that their exps are ready
            emit_pv(1, 2)
            for j in range(NKT):
                emit_scores(3, j)
                if j == 2:
                    emit_pv(1, 3)
                if j == 7:
                    emit_pv(3, 0)
                if j == 10:
                    emit_pv(3, 1)
                if j == 14:
                    emit_pv(3, 2)

            # -- h2 last: pv(3,3)/pv(2)/outproj fill the PE between pieces
            for j in range(NKT):
                emit_scores(2, j)
                if j == 1:
                    emit_pv(3, 3)
                if j == 4:
                    emit_pv(2, 0)
                if j == 6:
                    outproj_slice(0, split_dma=True)
                if j == 8:
                    emit_pv(2, 1)
                if j == 10:
                    outproj_slice(1, split_dma=True)
                if j == 12:
                    emit_pv(2, 2)
                if j == 14:
                    # pre-accumulate pv(2,3) over the k-tiles whose exps are
                    # already done; only j=12..15 remain after the last exp
                    emit_pv(2, 3, j_lo=0, j_hi=11, finish=False)
            # outproj(2) here: its matmuls fill the PE during the last two
            # h2 exps so the HAM clock never drops before the tail
            outproj_slice(2, cast="both", split_dma=True)
            emit_pv(2, 3, j_lo=12)
            outproj_slice(3, cast="both", split_dma=True)

    nc.compile()
    return nc


_PROGRAM = None


def _get_program():
    global _PROGRAM
    if _PROGRAM is None:
        _PROGRAM = build_program()
    return _PROGRAM


def _host_consts(token_positions):
    pos = np.asarray(token_positions, dtype=np.float32)
    inv = (
        ROPE_THETA ** (-np.arange(0, DH, 2, dtype=np.float32) / DH)
    ).astype(np.float32)
    ang = pos[:, None] * inv[None, :]  # [S, 32]
    cos, sin = np.cos(ang), np.sin(ang)
    rows = (np.arange(P) % DH) // 2
    cosT = np.ascontiguousarray(cos.T[rows]).astype(np.float16)
    sinT = np.ascontiguousarray(sin.T[rows]).astype(np.float16)
    Smat = np.zeros((P, P), dtype=np.float32)
    idx = np.arange(0, P, 2)
    Smat[idx, idx + 1] = -1.0
    Smat[idx + 1, idx] = 1.0
    STc = np.ascontiguousarray(Smat.T).astype(np.float16)
    tri = (np.arange(P)[None, :] >= np.arange(P)[:, None]).astype(np.float16)
    return cosT, sinT, STc, tri


def _make_in_maps(x, W_q, W_k, W_v, W_o, token_positions):
    cosT, sinT, STc, tri = _host_consts(token_positions)
    x = np.asarray(x, dtype=np.float32)
    maps = []
    for core in range(NCORE):
        b, hg = core // 4, core % 4
        hsl = slice(256 * hg, 256 * (hg + 1))
        # W_o columns for this core's local attention dims (its 4 heads);
        # each core emits a full [1024, 2048] partial that the host sums.
        wo_p = np.asarray(W_o, dtype=np.float32)[:, hsl].T   # [256 c, 1024 o]
        maps.append(
            {
                "xT": np.ascontiguousarray(x[b].T).astype(np.float16),
                "wqT": np.ascontiguousarray(np.asarray(W_q, np.float32)[hsl].T).astype(np.float16),
                "wkT": np.ascontiguousarray(np.asarray(W_k, np.float32)[hsl].T).astype(np.float16),
                "wvT": np.ascontiguousarray(np.asarray(W_v, np.float32)[hsl].T).astype(np.float16),
                "woT": np.ascontiguousarray(wo_p).astype(np.float16),
                "cosT": cosT,
                "sinT": sinT,
                "ST": STc,
                "trimask": tri,
            }
        )
    return maps


def _assemble(results):
    out = np.zeros((B, S, D), dtype=np.float32)
    for core in range(NCORE):
        b = core // 4
        out[b] += results[core]["outT"].astype(np.float32).T
    return out


def _run(in_maps, trace=False):
    nc = _get_program()
    tmpdir = None
    if trace:
        import tempfile

        tmpdir = tempfile.mkdtemp(prefix="ntff_", dir="/tmp")
    res = run_bass_kernel_spmd(
        nc, in_maps, list(range(NCORE)), trace=trace, tmpdir=tmpdir
    )
    return res


def kernel(x, W_q, W_k, W_v, W_o, token_positions):
    in_maps = _make_in_maps(x, W_q, W_k, W_v, W_o, token_positions)
    res = _run(in_maps)
    return _assemble(res.results)


def _install_profile_hook():
    """The agent image's antenv lacks axon_hooks; shim it so trace=True works."""
    import sys
    import types

    try:
        from antenv.axon_hooks import get_axon_ntff_profile_hook  # noqa: F401
        return
    except ImportError:
        pass
    import antenv
    from trn_agent_boot.trn_boot import _ntff_profile_via_ctypes

    mod = types.ModuleType("antenv.axon_hooks")
    _hook = {"h": None}
    mod.set_axon_ntff_profile_hook = lambda h: _hook.__setitem__("h", h)
    mod.get_axon_ntff_profile_hook = lambda: _hook["h"]
    sys.modules["antenv.axon_hooks"] = mod
    antenv.axon_hooks = mod
    mod.set_axon_ntff_profile_hook(
        _ntff_profile_via_ctypes("/opt/axon/libaxon_pjrt.so")
    )
    import concourse.bass_utils as bu

    bu.upload_artifacts = lambda d: f"file://{d}"


def kernel_traced(x, W_q, W_k, W_v, W_o, token_positions):
    """Returns (output, exec_time_ns, trace_path)."""
    _install_profile_hook()
    in_maps = _make_in_maps(x, W_q, W_k, W_v, W_o, token_positions)
    res = _run(in_maps, trace=True)
    trace_path = None
    if res.instructions_and_trace is not None:
        trace_path = res.instructions_and_trace[1]
    return _assemble(res.results), res.exec_time_ns, trace_path


# revision 55
# speedup vs baseline: 1.0073x; 1.0073x over previous
"""Causal multi-head self-attention with RoPE on 8 Trainium2 NeuronCores.

Sharding: data parallel over batch (2) x tensor parallel over heads (4 groups
of 4 heads).  Core c handles batch b = c // 4, head group hg = c % 4.

Per-core dataflow (everything stays in "transposed" [feature, seq] layouts so
no on-device transposes are ever needed):
  QT = wqT.T @ xT   [256, 2048]   (fp16 matmuls, d-contraction on partitions)
  RoPE via a constant shuffle matmul: rot = QT*cosT + (S @ (QT*sinT))
  V  = xT.T @ wvT   [2048, 256] -> fp16, augmented with 64 ones columns per
  head so the PV matmul emits the softmax denominator replicated on
  partitions 64..127 (fast [64,512] DVE reciprocal, no 1-lane ops)
  per head h:
    scores^T[ktile j] = Krot_h[:,128j:128j+128].T @ Qrot_h   (k on partitions)
    expP = exp(scores/8) fp16; diagonal block masked by a 0/1 tri mult (DVE)
    per 512-query chunk: psum[128,512] = sum_j V_aug_j.T @ expP_j
       rows 64..127 are the softmax denominator
    A^T = psum[0:64] * recip(psum[64:128])  -> fp16
  outT_partial = woT.T @ A^T  [1024, 2048] fp32 -> fp16; host sums partials

Scheduling (v4, ~172us vs 191us for v2): every input tensor is DMA'd in
dt-halves split across the sync+gpsimd queues so full-depth column slabs
land as early as possible; wv/x/wq lead so V tiles and the mt0 projections
start the moment data exists.  A dep-free dummy-matmul burst (plus small
packs woven between the first V/Q/K emissions) keeps the HAM clock gate at
8/8 across the DMA-only window.  h0's scores are emitted as single-chunk
[128,<=512] pieces that track the DMA ramp; h1-h3 use 1024-wide pieces.
Score psum rotates 3-deep (sA/sB/sC) and also hosts the rope shuffle psum
and the 1024-wide outproj psum pairs, so the PE is matmul-bound rather than
cast-bound.  pv work is placed in the scalar-bound h1/h3 stretches (pv(0),
pv(1), pv(3,0..2)) so the PE-bound h2 endgame carries only pv(2)/pv(3,3) +
outproj; pv(2,3) pre-accumulates k-tiles 0..11 before the last exp, and
outproj(2) runs after the final score piece so its matmuls cover the last
exp lull (no HAM drop before the tail).  psum->fp16 casts alternate between
the vector and scalar engines; output DMA goes out in 4-feature-tile pieces
alternating queues (finer on the last slice).  The engines execute their
queues in emission order, so the emission order below IS the schedule.
PE ~100% busy from ~30us to ~160us; remaining time is the DMA-bound ramp
(~21us of input at ~350GB/s + ~9us queue spin-up) and ~16us of fixed NEFF
preamble/epilogue.  Cross-partition-base DVE operand pairs fail on HW
(lanes have no cross-lane path) - the pv denominator is copied to base-0
partitions before reciprocal_approx_fast.
"""

import numpy as np

import concourse.bass as bass
import concourse.mybir as mybir
import concourse.tile as tile
from concourse import bacc
from concourse.bass_utils import run_bass_kernel_spmd

F32 = mybir.dt.float32
F16 = mybir.dt.float16

B, S, D, H, DH = 2, 2048, 1024, 16, 64
ROPE_THETA = 10000.0
NCORE = 8
HPG = 4          # heads per group (per core)
P = 128
NKT = S // P     # 16 k-tiles
NQC = S // 512   # 4 query chunks

# expP storage: k-tile j's columns start at global q = 128*j; width below.
_W = [S - P * j for j in range(NKT)]
_OFF = np.concatenate([[0], np.cumsum(_W)]).astype(int)
EXP_TOT = int(_OFF[-1])  # 17408 columns of fp16 -> 34KB/partition


def build_program():
    nc = bacc.Bacc(
        "TRN2", target_bir_lowering=False, debug=False, num_devices=NCORE
    )

    xT = nc.dram_tensor("xT", [D, S], F16, kind="ExternalInput")
    wqT = nc.dram_tensor("wqT", [D, 256], F16, kind="ExternalInput")
    wkT = nc.dram_tensor("wkT", [D, 256], F16, kind="ExternalInput")
    wvT = nc.dram_tensor("wvT", [D, 256], F16, kind="ExternalInput")
    woT = nc.dram_tensor("woT", [256, D], F16, kind="ExternalInput")
    cosT = nc.dram_tensor("cosT", [P, S], F16, kind="ExternalInput")
    sinT = nc.dram_tensor("sinT", [P, S], F16, kind="ExternalInput")
    ST = nc.dram_tensor("ST", [P, P], F16, kind="ExternalInput")
    trimask = nc.dram_tensor("trimask", [P, P], F16, kind="ExternalInput")

    outT = nc.dram_tensor("outT", [D, S], F16, kind="ExternalOutput")

    with tile.TileContext(nc) as tc:
        with (
            tc.tile_pool(name="const", bufs=1) as cpool,
            tc.tile_pool(name="qkv", bufs=1) as qkv,
            tc.tile_pool(name="psum", bufs=1, space="PSUM") as psum,
            tc.tile_pool(name="agp", bufs=4) as agp,
            tc.tile_pool(name="atmp", bufs=3) as atmp,
        ):
            tri_sb = cpool.tile([P, P], F16, tag="tri")
            wo_sb = cpool.tile([P, 2, D], F16, tag="wo")
            st_sb = cpool.tile([P, P], F16, tag="st")
            cos_sb = cpool.tile([P, S], F16, tag="cos")
            sin_sb = cpool.tile([P, S], F16, tag="sin")
            wq_sb = cpool.tile([P, 8, 256], F16, tag="wq")
            wk_sb = cpool.tile([P, 8, 256], F16, tag="wk")
            wv_sb = cpool.tile([P, 8, 256], F16, tag="wv")
            xt_sb = cpool.tile([P, 8, S], F16, tag="xt")

            qrot = qkv.tile([P, 2, S], F16, tag="qrot")
            krot = qkv.tile([P, 2, S], F16, tag="krot")
            # per k-tile, per head: [V (64) | ones (64)] so PV emits the
            # denominator replicated on partitions 64..127
            v_sb = qkv.tile([P, NKT, HPG, P], F16, tag="v")
            at_sb = qkv.tile([P, 2, S], F16, tag="at")
            eps = [
                qkv.tile([P, EXP_TOT], F16, tag=f"expp{i}", name=f"ep{i}")
                for i in range(2)
            ]
            # head 3's exp buffer recycles the x buffer: x is fully consumed
            # (mt1 projections + V) before h3's first score piece writes it
            eps.append(cpool.tile([P, EXP_TOT], F16, tag="xt", name="ep2"))
            EPMAP = {0: 0, 1: 1, 2: 0, 3: 2}

            # ---- PE warm-up: dummy matmuls spanning the DMA-only window so
            # the HAM clock gate is at 8/8 when the first real operands land.
            warm_w = cpool.tile([P, P], F16, tag="warm_w")
            warm_m = cpool.tile([P, 256], F16, tag="warm_m")
            nc.vector.memset(warm_w[:], 0.0)
            nc.vector.memset(warm_m[:], 0.0)
            # shares the sC slot: all weave points precede the first score
            # piece, so nothing live is clobbered and no cross-engine waits
            wp = psum.tile([P, 256], F32, tag="sC", bufs=1, name="warm")

            def emit_warm(n):
                # dep-free dummy matmuls: bridge DMA-wait gaps so the HAM
                # clock gate never sees an idle PE during the ramp
                for _ in range(n):
                    nc.tensor.matmul(
                        wp[:], warm_w[:], warm_m[:], start=True, stop=True
                    )

            emit_warm(48)

            # ---- input DMA: every tensor split in dt-halves across BOTH
            # queues (sync gets dt 0:4, gpsimd dt 4:8) so full-depth column
            # slabs complete as early as possible; consts interleaved.
            xr = xT.rearrange("(n p) m -> p n m", p=P)
            wvr = wvT.rearrange("(n p) m -> p n m", p=P)
            wqr = wqT.rearrange("(n p) m -> p n m", p=P)
            wkr = wkT.rearrange("(n p) m -> p n m", p=P)
            wor = woT.rearrange("(n p) m -> p n m", p=P)
            nc.sync.dma_start(out=wv_sb[:, 0:4], in_=wvr[:, 0:4])
            nc.gpsimd.dma_start(out=wv_sb[:, 4:8], in_=wvr[:, 4:8])
            nc.sync.dma_start(out=xt_sb[:, 0:4, 0:512], in_=xr[:, 0:4, 0:512])
            nc.gpsimd.dma_start(out=xt_sb[:, 4:8, 0:512], in_=xr[:, 4:8, 0:512])
            nc.sync.dma_start(out=wq_sb[:, 0:4], in_=wqr[:, 0:4])
            nc.gpsimd.dma_start(out=wq_sb[:, 4:8], in_=wqr[:, 4:8])
            nc.sync.dma_start(out=sin_sb[:, 0:1024], in_=sinT[:, 0:1024])
            nc.gpsimd.dma_start(out=st_sb[:], in_=ST[:, :])
            nc.gpsimd.dma_start(out=tri_sb[:], in_=trimask[:, :])
            nc.gpsimd.dma_start(out=cos_sb[:, 0:1024], in_=cosT[:, 0:1024])
            nc.sync.dma_start(out=wk_sb[:, 0:4], in_=wkr[:, 0:4])
            nc.gpsimd.dma_start(out=wk_sb[:, 4:8], in_=wkr[:, 4:8])
            nc.sync.dma_start(out=xt_sb[:, 0:4, 512:1024], in_=xr[:, 0:4, 512:1024])
            nc.gpsimd.dma_start(out=xt_sb[:, 4:8, 512:1024], in_=xr[:, 4:8, 512:1024])
            nc.sync.dma_start(out=xt_sb[:, 0:4, 1024:1536], in_=xr[:, 0:4, 1024:1536])
            nc.gpsimd.dma_start(out=xt_sb[:, 4:8, 1024:1536], in_=xr[:, 4:8, 1024:1536])
            nc.sync.dma_start(out=sin_sb[:, 1024:2048], in_=sinT[:, 1024:2048])
            nc.gpsimd.dma_start(out=cos_sb[:, 1024:2048], in_=cosT[:, 1024:2048])
            nc.sync.dma_start(out=xt_sb[:, 0:4, 1536:2048], in_=xr[:, 0:4, 1536:2048])
            nc.gpsimd.dma_start(out=xt_sb[:, 4:8, 1536:2048], in_=xr[:, 4:8, 1536:2048])
            nc.sync.dma_start(out=wo_sb[:, 0:1], in_=wor[:, 0:1])
            nc.gpsimd.dma_start(out=wo_sb[:, 1:2], in_=wor[:, 1:2])

            # ---------------- emission helpers -----------------
            def emit_proj(mt, sc, which):
                # one projection (q or k) + rope for one (mt, sc) pair
                ssl = bass.ts(sc, 512)
                w_sb, rot = (
                    (wq_sb, qrot) if which == "q" else (wk_sb, krot)
                )
                pp = psum.tile([P, 512], F32, tag="proj", bufs=2)
                for dt in range(8):
                    nc.tensor.matmul(
                        pp[:],
                        w_sb[:, dt, P * mt:P * (mt + 1)],
                        xt_sb[:, dt, ssl],
                        start=(dt == 0),
                        stop=(dt == 7),
                    )
                t_s = atmp.tile([P, 512], F16, tag="ts", bufs=3)
                nc.vector.tensor_tensor(
                    out=t_s[:], in0=pp[:], in1=sin_sb[:, ssl],
                    op=mybir.AluOpType.mult,
                )
                sh = _score_psum(512)
                nc.tensor.matmul(
                    sh[:], st_sb[:], t_s[:], start=True, stop=True
                )
                nc.vector.tensor_tensor(
                    out=rot[:, mt, ssl], in0=pp[:],
                    in1=cos_sb[:, ssl], op=mybir.AluOpType.mult,
                )
                nc.vector.tensor_tensor(
                    out=rot[:, mt, ssl], in0=rot[:, mt, ssl],
                    in1=sh[:], op=mybir.AluOpType.add,
                )

            def emit_v(st):
                # V projection for one 128-row seq tile -> fp16 V_aug
                vp = psum.tile([P, 256], F32, tag="proj", bufs=2, name="vp")
                for dt in range(8):
                    nc.tensor.matmul(
                        vp[:],
                        xt_sb[:, dt, P * st:P * (st + 1)],
                        wv_sb[:, dt, :],
                        start=(dt == 0),
                        stop=(dt == 7),
                    )
                # first tiles cast on the DVE (idle before the rope chains
                # begin); later ones on the scalar engine (idle pre-exp) so
                # the DVE stays out of the mid-ramp critical path
                if st < 4:
                    nc.vector.tensor_copy(
                        out=v_sb[:, st, :, 0:DH],
                        in_=vp.rearrange("p (h d) -> p h d", h=HPG),
                    )
                else:
                    nc.scalar.activation(
                        out=v_sb[:, st, :, 0:DH],
                        in_=vp.rearrange("p (h d) -> p h d", h=HPG),
                        func=mybir.ActivationFunctionType.Copy,
                    )

            piece_ctr = [0]

            def _score_psum(width):
                tag = ("sA", "sB", "sC")[piece_ctr[0] % 3]
                piece_ctr[0] += 1
                return psum.tile([P, width], F32, tag=tag, bufs=1, name="sp")

            def emit_piece(h, j, c, tri="defer"):
                # single-chunk score piece: head h, k-tile j, query chunk c
                ep = eps[EPMAP[h]]
                th, bs = h // 2, 64 * (h % 2)
                qh = qrot[bs:bs + 64, th, :]
                kh = krot[bs:bs + 64, th, :]
                off = int(_OFF[j])
                r = j % 4
                ls = P * r if j // 4 == c else 0
                q0 = 512 * c + ls
                w = 512 * (c + 1) - q0
                sp = _score_psum(512)
                nc.tensor.matmul(
                    sp[:, ls:ls + w],
                    kh[:, P * j:P * (j + 1)],
                    qh[:, q0:q0 + w],
                    start=True, stop=True,
                )
                nc.scalar.activation(
                    out=ep[:, off + q0 - P * j:off + q0 - P * j + w],
                    in_=sp[:, ls:ls + w],
                    func=mybir.ActivationFunctionType.Exp,
                    scale=0.125,
                )
                if j // 4 == c:
                    if tri == "defer":
                        # deferred out of the DVE-choked ramp; flushed (on
                        # gpsimd) before the first pv consumer
                        deferred_tri.append((ep, off))
                    else:
                        nc.vector.tensor_tensor(
                            out=ep[:, off:off + P], in0=ep[:, off:off + P],
                            in1=tri_sb[:], op=mybir.AluOpType.mult,
                        )

            deferred_tri = []

            def flush_tri():
                # on gpsimd: idle after input DMA issue, and this keeps the
                # DVE out of the ramp critical path
                for ep, off in deferred_tri:
                    nc.gpsimd.tensor_tensor(
                        out=ep[:, off:off + P], in0=ep[:, off:off + P],
                        in1=tri_sb[:], op=mybir.AluOpType.mult,
                    )
                del deferred_tri[:]

            def emit_scores(h, j):
                # all remaining chunks for (h, j) in up-to-1024-wide pieces
                ep = eps[EPMAP[h]]
                th, bs = h // 2, 64 * (h % 2)
                qh = qrot[bs:bs + 64, th, :]
                kh = krot[bs:bs + 64, th, :]
                c0, r = j // 4, j % 4
                off = int(_OFF[j])
                cs = list(range(c0, 4))
                for gi in range(0, len(cs), 2):
                    grp = cs[gi:gi + 2]
                    ca = grp[0]
                    sp = _score_psum(1024)
                    for c in grp:
                        loc = 512 * (c - ca)
                        if c == c0:
                            nc.tensor.matmul(
                                sp[:, loc + 128 * r:loc + 512],
                                kh[:, P * j:P * (j + 1)],
                                qh[:, 512 * c + 128 * r:512 * (c + 1)],
                                start=True, stop=True,
                            )
                        else:
                            nc.tensor.matmul(
                                sp[:, loc:loc + 512],
                                kh[:, P * j:P * (j + 1)],
                                qh[:, 512 * c:512 * (c + 1)],
                                start=True, stop=True,
                            )
                    ls = 128 * r if ca == c0 else 0
                    qstart = 512 * ca + ls
                    w = 512 * (grp[-1] + 1) - qstart
                    eo = off + qstart - 128 * j
                    nc.scalar.activation(
                        out=ep[:, eo:eo + w],
                        in_=sp[:, ls:ls + w],
                        func=mybir.ActivationFunctionType.Exp,
                        scale=0.125,
                    )
                    if gi == 0:
                        nc.vector.tensor_tensor(
                            out=ep[:, off:off + P], in0=ep[:, off:off + P],
                            in1=tri_sb[:], op=mybir.AluOpType.mult,
                        )

            pv_pend = {}

            def emit_pv(h, c, j_lo=0, j_hi=None, finish=True):
                ep = eps[EPMAP[h]]
                th, bs = h // 2, 64 * (h % 2)
                last_j = 4 * c + 3
                if j_hi is None:
                    j_hi = last_j
                if (h, c) in pv_pend:
                    pv = pv_pend.pop((h, c))
                else:
                    pv = psum.tile([P, 512], F32, tag="proj", bufs=2, name="pv")
                for j in range(j_lo, j_hi + 1):
                    off = int(_OFF[j])
                    if j // 4 == c:
                        rr = j % 4
                        n = 512 - 128 * rr
                        nc.tensor.matmul(
                            pv[:, 128 * rr:512],
                            v_sb[:, j, h, :],
                            ep[:, off:off + n],
                            start=(j == 0), stop=(j == last_j),
                        )
                    else:
                        st_col = off + 512 * c - 128 * j
                        nc.tensor.matmul(
                            pv[:, :],
                            v_sb[:, j, h, :],
                            ep[:, st_col:st_col + 512],
                            start=(j == 0), stop=(j == last_j),
                        )
                if not finish:
                    pv_pend[(h, c)] = pv
                    return
                den = atmp.tile([DH, 512], F32, tag="den", bufs=2)
                nc.vector.tensor_copy(out=den[:], in_=pv[DH:P, :])
                recip = atmp.tile([DH, 512], F32, tag="recip", bufs=2)
                nc.vector.reciprocal_approx_fast(out=recip[:], in_=den[:])
                nc.vector.tensor_tensor(
                    out=at_sb[bs:bs + 64, th, 512 * c:512 * (c + 1)],
                    in0=pv[0:DH, :], in1=recip[:],
                    op=mybir.AluOpType.mult,
                )

            outTr = outT.rearrange("(n p) m -> p n m", p=P)

            def outproj_slice(sc, cast="vector", split_dma=False):
                # outT_partial[:, sc] = sum over the 256 LOCAL attention
                # dims (this core's 4 heads); host sums the partials.
                # Two output-feature tiles per [P,1024] psum (on the 3-deep
                # score rotation, so the PE is matmul-bound, not cast-bound);
                # one cast per pair; one DMA per 4 tiles, alternating queues.
                ssl = bass.ts(sc, 512)
                for grp in range(2):
                    ob = agp.tile([P, 4, 512], F16, tag="ob", name="ob")
                    for half in range(2):
                        po = _score_psum(1024)
                        for oi in range(2):
                            ot = 4 * grp + 2 * half + oi
                            osl = bass.ts(ot, P)
                            for ct in range(2):
                                nc.tensor.matmul(
                                    po[:, 512 * oi:512 * (oi + 1)],
                                    wo_sb[:, ct, osl],
                                    at_sb[:, ct, ssl],
                                    start=(ct == 0), stop=(ct == 1),
                                )
                        use_scalar = (
                            cast == "scalar"
                            or (cast == "both" and half % 2 == 1)
                        )
                        if use_scalar:
                            nc.scalar.activation(
                                out=ob[:, 2 * half:2 * half + 2],
                                in_=po.rearrange("p (a b) -> p a b", a=2),
                                func=mybir.ActivationFunctionType.Copy,
                            )
                        else:
                            nc.vector.tensor_copy(
                                out=ob[:, 2 * half:2 * half + 2],
                                in_=po.rearrange("p (a b) -> p a b", a=2),
                            )
                        if split_dma:
                            # finer pieces for the kernel tail: both queues
                            # stream the final slice out in parallel
                            o0 = 4 * grp + 2 * half
                            q = nc.sync if (2 * grp + half) % 2 == 0 else nc.gpsimd
                            q.dma_start(
                                out=outTr[:, o0:o0 + 2, ssl],
                                in_=ob[:, 2 * half:2 * half + 2],
                            )
                    if split_dma:
                        continue
                    if grp == 0:
                        nc.sync.dma_start(
                            out=outTr[:, 0:4, ssl], in_=ob[:]
                        )
                    else:
                        nc.gpsimd.dma_start(
                            out=outTr[:, 4:8, ssl], in_=ob[:]
                        )

            # ---------------- emission schedule -----------------
            # Engines execute their queues in emission order, so this order IS
            # the schedule.  Constraints: the scalar-engine exp stream must
            # start ASAP and never starve; the PE must always have
            # exp-independent filler (V proj, projections, pv of older heads)
            # between score pieces; ep buffers are shared h0/h2, so all pv(0)
            # precede h2 scores; ep2 aliases x, so h3 scores follow the last
            # x consumer (V15 / mt1 sc3 projections).

            # -- ramp (DMA-gated): V tiles as x columns land, mt0 Q/K,
            # granular h0 score pieces chunk-by-chunk
            emit_v(0)
            emit_warm(6)
            emit_v(1)
            emit_warm(6)
            emit_proj(0, 0, "q")
            emit_warm(6)
            emit_v(2)
            emit_warm(4)
            emit_v(3)
            emit_warm(4)
            emit_proj(0, 0, "k")
            for j in range(4):
                emit_piece(0, j, 0)
            emit_proj(0, 1, "q")
            emit_proj(0, 1, "k")
            emit_piece(0, 0, 1)
            emit_piece(0, 1, 1)
            emit_v(4)
            emit_piece(0, 2, 1)
            emit_piece(0, 3, 1)
            emit_v(5)
            emit_piece(0, 4, 1)
            emit_piece(0, 5, 1)
            emit_v(6)
            emit_piece(0, 6, 1)
            emit_piece(0, 7, 1)
            emit_v(7)
            emit_proj(0, 2, "q")
            emit_proj(0, 2, "k")
            emit_piece(0, 0, 2)
            emit_piece(0, 1, 2)
            emit_v(8)
            emit_piece(0, 2, 2)
            emit_piece(0, 3, 2)
            emit_v(9)
            emit_piece(0, 4, 2)
            emit_piece(0, 5, 2)
            emit_v(10)
            emit_piece(0, 6, 2)
            emit_piece(0, 7, 2)
            emit_v(11)
            emit_piece(0, 8, 2)
            emit_piece(0, 9, 2)
            emit_proj(0, 3, "q")
            emit_piece(0, 10, 2)
            emit_piece(0, 11, 2)
            emit_proj(0, 3, "k")
            emit_piece(0, 0, 3)
            emit_piece(0, 1, 3)
            emit_v(12)
            emit_piece(0, 2, 3)
            emit_piece(0, 3, 3)
            emit_v(13)
            emit_piece(0, 4, 3)
            emit_piece(0, 5, 3)
            emit_v(14)
            emit_piece(0, 6, 3)
            emit_piece(0, 7, 3)
            emit_v(15)
            emit_piece(0, 8, 3)
            emit_piece(0, 9, 3)
            emit_proj(1, 0, "q")
            emit_piece(0, 10, 3)
            emit_piece(0, 11, 3)
            emit_proj(1, 0, "k")
            emit_piece(0, 12, 3)
            emit_piece(0, 13, 3)
            emit_proj(1, 1, "q")
            emit_piece(0, 14, 3)
            emit_piece(0, 15, 3)
            emit_proj(1, 1, "k")

            # -- h1 pieces + rest of mt1 + pv(0); deferred DVE housekeeping
            # (h0 diagonal masks + the V ones-columns) runs here where the
            # vector engine has slack
            nc.gpsimd.memset(v_sb[:, :, :, DH:P], 1.0)
            flush_tri()
            emit_scores(1, 0)
            emit_proj(1, 2, "q")
            emit_scores(1, 1)
            emit_proj(1, 2, "k")
            emit_scores(1, 2)
            emit_pv(0, 0)
            emit_scores(1, 3)
            emit_proj(1, 3, "q")
            emit_scores(1, 4)
            emit_proj(1, 3, "k")
            emit_scores(1, 5)
            emit_pv(0, 1)
            emit_scores(1, 6)
            emit_scores(1, 7)
            emit_pv(0, 2)
            emit_scores(1, 8)
            emit_scores(1, 9)
            emit_pv(0, 3)
            emit_scores(1, 10)
            emit_scores(1, 11)
            emit_pv(1, 0)
            emit_scores(1, 12)
            emit_scores(1, 13)
            emit_pv(1, 1)
            emit_scores(1, 14)
            emit_scores(1, 15)

            # -- h3 pieces (own ep buffer = x alias; x fully consumed);
            # h3 is scalar-bound, so it absorbs pv(1,2/3) and pv(3,0..2)
            for j in range(NKT):
                emit_scores(3, j)
                if j == 2:
                    emit_pv(1, 2)
                if j == 5:
                    emit_pv(1, 3)
                if j == 8:
                    emit_pv(3, 0)
                if j == 11:
                    emit_pv(3, 1)
                if j == 14:
                    emit_pv(3, 2)

            # -- h2 last: pv(3,3)/pv(2)/outproj fill the PE between pieces
            for j in range(NKT):
                emit_scores(2, j)
                if j == 1:
                    emit_pv(3, 3)
                if j == 4:
                    emit_pv(2, 0)
                if j == 6:
                    outproj_slice(0)
                if j == 8:
                    emit_pv(2, 1)
                if j == 10:
                    outproj_slice(1)
                if j == 12:
                    emit_pv(2, 2)
                if j == 14:
                    # pre-accumulate pv(2,3) over the k-tiles whose exps are
                    # already done; only j=12..15 remain after the last exp
                    emit_pv(2, 3, j_lo=0, j_hi=11, finish=False)
            # outproj(2) here: its matmuls fill the PE during the last two
            # h2 exps so the HAM clock never drops before the tail
            outproj_slice(2, cast="both", split_dma=True)
            emit_pv(2, 3, j_lo=12)
            outproj_slice(3, cast="both", split_dma=True)

    nc.compile()
    return nc


_PROGRAM = None


def _get_program():
    global _PROGRAM
    if _PROGRAM is None:
        _PROGRAM = build_program()
    return _PROGRAM


def _host_consts(token_positions):
    pos = np.asarray(token_positions, dtype=np.float32)
    inv = (
        ROPE_THETA ** (-np.arange(0, DH, 2, dtype=np.float32) / DH)
    ).astype(np.float32)
    ang = pos[:, None] * inv[None, :]  # [S, 32]
    cos, sin = np.cos(ang), np.sin(ang)
    rows = (np.arange(P) % DH) // 2
    cosT = np.ascontiguousarray(cos.T[rows]).astype(np.float16)
    sinT = np.ascontiguousarray(sin.T[rows]).astype(np.float16)
    Smat = np.zeros((P, P), dtype=np.float32)
    idx = np.arange(0, P, 2)
    Smat[idx, idx + 1] = -1.0
    Smat[idx + 1, idx] = 1.0
    STc = np.ascontiguousarray(Smat.T).astype(np.float16)
    tri = (np.arange(P)[None, :] >= np.arange(P)[:, None]).astype(np.float16)
    return cosT, sinT, STc, tri


def _make_in_maps(x, W_q, W_k, W_v, W_o, token_positions):
    cosT, sinT, STc, tri = _host_consts(token_positions)
    x = np.asarray(x, dtype=np.float32)
    maps = []
    for core in range(NCORE):
        b, hg = core // 4, core % 4
        hsl = slice(256 * hg, 256 * (hg + 1))
        # W_o columns for this core's local attention dims (its 4 heads);
        # each core emits a full [1024, 2048] partial that the host sums.
        wo_p = np.asarray(W_o, dtype=np.float32)[:, hsl].T   # [256 c, 1024 o]
        maps.append(
            {
                "xT": np.ascontiguousarray(x[b].T).astype(np.float16),
                "wqT": np.ascontiguousarray(np.asarray(W_q, np.float32)[hsl].T).astype(np.float16),
                "wkT": np.ascontiguousarray(np.asarray(W_k, np.float32)[hsl].T).astype(np.float16),
                "wvT": np.ascontiguousarray(np.asarray(W_v, np.float32)[hsl].T).astype(np.float16),
                "woT": np.ascontiguousarray(wo_p).astype(np.float16),
                "cosT": cosT,
                "sinT": sinT,
                "ST": STc,
                "trimask": tri,
            }
        )
    return maps


def _assemble(results):
    out = np.zeros((B, S, D), dtype=np.float32)
    for core in range(NCORE):
        b = core // 4
        out[b] += results[core]["outT"].astype(np.float32).T
    return out


def _run(in_maps, trace=False):
    nc = _get_program()
    tmpdir = None
    if trace:
        import tempfile

        tmpdir = tempfile.mkdtemp(prefix="ntff_", dir="/tmp")
    res = run_bass_kernel_spmd(
        nc, in_maps, list(range(NCORE)), trace=trace, tmpdir=tmpdir
    )
    return res


def kernel(x, W_q, W_k, W_v, W_o, token_positions):
    in_maps = _make_in_maps(x, W_q, W_k, W_v, W_o, token_positions)
    res = _run(in_maps)
    return _assemble(res.results)


def _install_profile_hook():
    """The agent image's antenv lacks axon_hooks; shim it so trace=True works."""
    import sys
    import types

    try:
        from antenv.axon_hooks import get_axon_ntff_profile_hook  # noqa: F401
        return
    except ImportError:
        pass
    import antenv
    from trn_agent_boot.trn_boot import _ntff_profile_via_ctypes

    mod = types.ModuleType("antenv.axon_hooks")
    _hook = {"h": None}
    mod.set_axon_ntff_profile_hook = lambda h: _hook.__setitem__("h", h)
    mod.get_axon_ntff_profile_hook = lambda: _hook["h"]
    sys.modules["antenv.axon_hooks"] = mod
    antenv.axon_hooks = mod
    mod.set_axon_ntff_profile_hook(
        _ntff_profile_via_ctypes("/opt/axon/libaxon_pjrt.so")
    )
    import concourse.bass_utils as bu

    bu.upload_artifacts = lambda d: f"file://{d}"


def kernel_traced(x, W_q, W_k, W_v, W_o, token_positions):
    """Returns (output, exec_time_ns, trace_path)."""
    _install_profile_hook()
    in_maps = _make_in_maps(x, W_q, W_k, W_v, W_o, token_positions)
    res = _run(in_maps, trace=True)
    trace_path = None
    if res.instructions_and_trace is not None:
        trace_path = res.instructions_and_trace[1]
    return _assemble(res.results), res.exec_time_ns, trace_path
